# revision 17
# baseline (speedup 1.0000x reference)
"""Multi-head causal attention (B=8, T=1024, C=1024, H=16, hs=64) on 8 trn2 cores.

Data-parallel over batch: core b computes full attention for x[b].

Device algorithm (per core), all matmuls bf16 inputs / fp32 PSUM accum:
  - xT [C, T] resident in SBUF (host pre-transposed, bf16), DMA'd per chunk
    interleaved with pair-0 Q/K weights so the PE starts ~1us in.
  - prologue: Q/K projection for pair 0 paced by the chunk DMAs, then the
    V projection for all heads (with a ones column at index 64 so the AV
    matmul also produces softmax denominators), interleaved with pair-0
    score blocks so ACT gets an early start.
  - scores transposed per (s_tile, head): scT[s_tile, t] = kT_chunk^T @ qT
    into a private [128,1024] PSUM tile, exp fused on ScalarE as ONE
    activation over the causal span (scale=1/8), diagonal 128x128 block
    masked by a tril multiply on GpSimd (keeps DVE/ACT free).
  - software-pipelined pair loop: iter p runs QK proj of pair p+2
    interleaved with score blocks of pair p+1 (covers PSUM-slot waits on
    the exp drain), then AV of pair p.
  - out^T[65, t] accumulated over s chunks: lhsT = [v | 1], rhs = expT.
    Row 64 = sum(exp) = softmax denominator. Normalize: copy PSUM->SBUF,
    DVE reciprocal on the [1,512] denominator row, GpSimd broadcast +
    multiply, DMA out.
"""

import numpy as np
import ml_dtypes

import concourse.bass as bass
import concourse.mybir as mybir
from concourse import bacc
from concourse.tile import TileContext
from concourse.bass import ds, ts
from concourse.bass_utils import run_bass_kernel_spmd
from concourse.masks import make_upper_triangular

BF16 = mybir.dt.bfloat16
F32 = mybir.dt.float32

B, T, C, H, HS = 8, 1024, 1024, 16, 64
P = 128
CK = C // P       # 8 contraction chunks
TT = T // P       # 8 t tiles
PAIRS = H // 2    # 8 head pairs
HALF = 512

_BUILT = None


def build_nc():
    nc = bacc.Bacc("TRN2", target_bir_lowering=False, debug=False)
    # [p, c, t] : xT[C, T] chunked; partition p, chunk c -> row 128c+p of xT
    xt = nc.dram_tensor("xt", [P, CK, T], BF16, kind="ExternalInput")
    # [proj(q,k), pair, p, c, f] : lhsT chunks, f = 2 heads x 64 stacked
    wqk = nc.dram_tensor("wqk", [2, PAIRS, P, CK, P], BF16, kind="ExternalInput")
    # [p, c, pair, f]
    wv = nc.dram_tensor("wv", [P, CK, PAIRS, P], BF16, kind="ExternalInput")
    # out^T per head: [head, d, t]; host transposes to [T, H*HS]
    out = nc.dram_tensor("out", [H, HS, T], F32, kind="ExternalOutput")

    with TileContext(nc) as tc:
        with (
            tc.tile_pool(name="const", bufs=1) as constp,
            tc.tile_pool(name="wpool", bufs=6) as wpool,
            tc.tile_pool(name="qkpool", bufs=6) as qkp,
            tc.tile_pool(name="exppool", bufs=16) as expp,
            tc.tile_pool(name="smallpool", bufs=4) as smallp,
            tc.tile_pool(name="ps1", bufs=2, space="PSUM") as ps1,
            tc.tile_pool(name="psSc", bufs=3, space="PSUM") as psSc,
        ):
            xt_sb = constp.tile([P, CK, T], BF16)
            # pair-group-major: [p, c, pg, 4*128] so the rhs slice is 2D
            wv_sb = constp.tile([P, CK, 2, 4 * P], BF16)

            # pair-0 weights first (2 whole DMAs), then xt per chunk on the
            # Sync queue; wv streams concurrently on the ACT queue so the V
            # phase isn't serialized behind xt.
            w_sb = {}
            w_sb[0] = (wpool.tile([P, CK, P], BF16, tag="w", name="wq0"),
                       wpool.tile([P, CK, P], BF16, tag="w", name="wk0"))
            nc.sync.dma_start(w_sb[0][0][:, :, :], wqk[0, 0, :, :, :])
            nc.sync.dma_start(w_sb[0][1][:, :, :], wqk[1, 0, :, :, :])
            for c in range(CK):
                nc.sync.dma_start(xt_sb[:, c, :], xt[:, c, :])
            for c in range(CK):
                nc.scalar.dma_start(
                    wv_sb[:, c, :, :],
                    wv[:, c, :, :].rearrange("p (g r) f -> p g (r f)", g=2),
                )

            def dma_w(pair):
                wq = wpool.tile([P, CK, P], BF16, tag="w", name=f"wq{pair}")
                wk = wpool.tile([P, CK, P], BF16, tag="w", name=f"wk{pair}")
                nc.sync.dma_start(wq[:, :, :], wqk[0, pair, :, :, :])
                nc.sync.dma_start(wk[:, :, :], wqk[1, pair, :, :, :])
                w_sb[pair] = (wq, wk)

            dma_w(1)
            dma_w(2)

            mask = constp.tile([P, P], BF16)
            make_upper_triangular(nc, mask, val=1.0, diag=True)
            # duplicated tril for masking both heads' diag blocks in one op
            mask2 = constp.tile([P, 2, P], BF16)
            for _w in range(2):
                nc.gpsimd.tensor_copy(mask2[:, _w, :], mask[:, :])
            # [s_p, head, s_tile, 64 v cols + 1 ones col]
            v_all = constp.tile([P, H, TT, HS + 1], BF16)
            nc.gpsimd.memset(v_all[:, :, :, HS:HS + 1], 1.0)

            qk = {}

            def qk_half(pair, half_idx):
                """One of 4 projection halves: 8 chunk matmuls + PSUM->SBUF cast.
                half_idx: 0=q g0, 1=q g1, 2=k g0, 3=k g1."""
                if pair not in qk:
                    qk[pair] = (qkp.tile([P, T], BF16, tag="qk", name=f"q{pair}"),
                                qkp.tile([P, T], BF16, tag="qk", name=f"k{pair}"))
                proj, g = divmod(half_idx, 2)
                wsb = w_sb[pair][proj]
                dst = qk[pair][proj]
                pp = ps1.tile([P, HALF], F32, tag="ps", name=f"pp{pair}_{half_idx}")
                for c in range(CK):
                    nc.tensor.matmul(
                        pp[:, :],
                        wsb[:, c, :],
                        xt_sb[:, c, ds(HALF * g, HALF)],
                        start=(c == 0),
                        stop=(c == CK - 1),
                    )
                nc.vector.tensor_copy(dst[:, ds(HALF * g, HALF)], pp[:, :])

            es = {}

            def sc_block(pair, i):
                """Scores + exp + causal mask for s-tile i, both heads.

                Per head: private [128, T] PSUM tile, matmuls for the causal
                span [(t0,512),(512,1024)] (or single span when t0>=512),
                then ONE exp over [t0, 1024) and a tril mask-mult on GpSimd
                for the diagonal block. The two heads' matmuls alternate so
                they pair up in PE row groups (0,0)/(64,0)."""
                qT, kT = qk[pair]
                t0 = P * i
                e2 = expp.tile([P, 2, T], BF16, tag="exp", name=f"e{pair}_{i}")
                es[(pair, i)] = e2
                scs = []
                for w in range(2):
                    scs.append(psSc.tile([P, T], F32, tag="sc",
                                         name=f"sc{pair}_{i}_{w}"))
                spans = [(t0, HALF), (HALF, T)] if t0 < HALF else [(t0, T)]
                for a, b in spans:
                    for w in range(2):
                        po = HS * w
                        nc.tensor.matmul(
                            scs[w][:, ds(a, b - a)],
                            kT[ds(po, HS), ds(t0, P)],
                            qT[ds(po, HS), ds(a, b - a)],
                        )
                for w in range(2):
                    nc.scalar.activation(
                        e2[:, w, ds(t0, T - t0)],
                        scs[w][:, ds(t0, T - t0)],
                        mybir.ActivationFunctionType.Exp,
                        scale=HS ** -0.5,
                    )
                # single tril mask-mult covering both heads' diagonal blocks
                nc.gpsimd.tensor_tensor(
                    e2[:, :, ds(t0, P)], e2[:, :, ds(t0, P)],
                    mask2[:, :, :], mybir.AluOpType.mult,
                )

            def av_mm(pair, hh, w):
                """AV accumulation for head 2*pair+w, cols [512*hh, +512)."""
                h = 2 * pair + w
                av = ps1.tile([HS + 1, HALF], F32, tag="ps", name=f"av{h}_{hh}")
                contrib = [i for i in range(TT) if P * i < HALF * (hh + 1)]
                for idx, i in enumerate(contrib):
                    g0 = max(HALF * hh, P * i)
                    g1 = HALF * (hh + 1)
                    nc.tensor.matmul(
                        av[:, ds(g0 - HALF * hh, g1 - g0)],
                        v_all[:, h, i, :],
                        es[(pair, i)][:, w, ds(g0, g1 - g0)],
                        start=(idx == 0),
                        stop=(idx == len(contrib) - 1),
                    )
                return av

            def av_normalize(pair, avls):
                """Stage-batched normalization of the pair's 4 AV halves.

                Per half: copy PSUM->SBUF, DMA-repartition the [1,512]
                denominator row to [128,4] (DVE reciprocal cost scales with
                the FREE size, so spread values across lanes), reciprocal,
                DMA back, GpSimd broadcast across partitions, DVE multiply,
                DMA out. All DMAs issue from the DVE queue right after their
                producers (no semaphore stalls on the Sync queue); stages
                are batched across the 4 halves so DMA latency hides behind
                the other halves' DVE work."""
                chains = []
                for (hh, w), av in avls:
                    h = 2 * pair + w
                    nm = f"{h}_{hh}"
                    avs = smallp.tile([HS + 1, HALF], F32, tag="avs",
                                      name=f"avs{nm}")
                    nc.vector.tensor_copy(avs[:, :], av[:, :])
                    den_t = smallp.tile([P, 4], F32, tag="dent",
                                        name=f"den{nm}")
                    nc.sync.dma_start(den_t[:, :], avs[HS:HS + 1, :])
                    chains.append((hh, w, h, nm, avs, den_t))
                rbs = []
                for hh, w, h, nm, avs, den_t in chains:
                    rec_t = smallp.tile([P, 4], F32, tag="rect",
                                        name=f"rec{nm}")
                    nc.vector.reciprocal(rec_t[:, :], den_t[:, :])
                    recip_row = smallp.tile([1, HALF], F32, tag="recip",
                                            name=f"recip{nm}")
                    nc.sync.dma_start(recip_row[:, :], rec_t[:, :])
                    rb = smallp.tile([HS, HALF], F32, tag="rb", name=f"rb{nm}")
                    nc.gpsimd.partition_broadcast(rb[:, :], recip_row[0:1, :])
                    rbs.append(rb)
                for (hh, w, h, nm, avs, den_t), rb in zip(chains, rbs):
                    osb = smallp.tile([HS, HALF], F32, tag="osb",
                                      name=f"osb{nm}")
                    nc.vector.tensor_tensor(
                        osb[:, :], avs[0:HS, :], rb[:, :], mybir.AluOpType.mult,
                    )
                    nc.sync.dma_start(out[h, :, ds(HALF * hh, HALF)],
                                        osb[:, :])

            def av_half(pair, hh, w, defer=None):
                av = av_mm(pair, hh, w)
                defer.append(((hh, w), av))

            # ---- prologue: QK proj pair 0 (paced by the xt chunk DMAs) ----
            for k in range(4):
                qk_half(0, k)

            # ---- V for all heads, pair-0 score blocks interleaved ----
            for j in range(TT):
                for pg in range(2):
                    pv = ps1.tile([P, HALF], F32, tag="ps", name=f"pv{j}_{pg}")
                    for c in range(CK):
                        nc.tensor.matmul(
                            pv[:, :],
                            xt_sb[:, c, ts(j, P)],
                            wv_sb[:, c, pg, :],
                            start=(c == 0),
                            stop=(c == CK - 1),
                        )
                    # pv cols are (head0..head7 of group) x 64 in order
                    nc.vector.tensor_copy(
                        v_all[:, ds(8 * pg, 8), j, 0:HS],
                        pv.rearrange("p (g d) -> p g d", d=HS),
                    )
                sc_block(0, j)

            # ---- QK proj pair 1 ----
            for k in range(4):
                qk_half(1, k)

            # ---- software-pipelined pair loop ----
            for p in range(PAIRS):
                if p + 3 < PAIRS:
                    dma_w(p + 3)
                avls = []
                if p + 2 < PAIRS:
                    # QK proj of pair p+2 interleaved with scores of pair p+1
                    for k in range(4):
                        qk_half(p + 2, k)
                        sc_block(p + 1, 2 * k)
                        sc_block(p + 1, 2 * k + 1)
                    for hh in range(2):
                        for w in range(2):
                            av_half(p, hh, w, defer=avls)
                elif p + 1 < PAIRS:
                    # p == 6: no QK8; interleave SC7 with AV6 instead
                    order = [("av", 0, 0), ("sc", 0, 1), ("av", 0, 1),
                             ("sc", 2, 3), ("av", 1, 0), ("sc", 4, 5),
                             ("av", 1, 1), ("sc", 6, 7)]
                    for kind, a0, a1 in order:
                        if kind == "av":
                            av_half(p, a0, a1, defer=avls)
                        else:
                            sc_block(p + 1, a0)
                            sc_block(p + 1, a1)
                else:
                    for hh in range(2):
                        for w in range(2):
                            av_half(p, hh, w, defer=avls)
                av_normalize(p, avls)
    nc.compile()
    return nc


def get_nc():
    global _BUILT
    if _BUILT is None:
        _BUILT = build_nc()
    return _BUILT


def prep_inputs(x, Wq, Wk, Wv):
    """Host-side shard + layout prep. Returns in_maps (one dict per core)."""
    x = np.asarray(x, dtype=np.float32)
    Wq = np.asarray(Wq, dtype=np.float32)
    Wk = np.asarray(Wk, dtype=np.float32)
    Wv = np.asarray(Wv, dtype=np.float32)
    bf = ml_dtypes.bfloat16

    # xT[b]: [C, T] -> [p, c, t] with row 128c+p
    xts = []
    for b in range(B):
        xT = np.ascontiguousarray(x[b].T)          # [C, T]
        xts.append(xT.reshape(CK, P, T).transpose(1, 0, 2).astype(bf))

    def pack_pairs(W):
        # [H, C, hs] -> [pair, C, 128] -> [pair, p, c, f]
        Wp = W.reshape(PAIRS, 2, C, HS).transpose(0, 2, 1, 3).reshape(PAIRS, C, P)
        return Wp.reshape(PAIRS, CK, P, P).transpose(0, 2, 1, 3)  # [pair, p, c, f]

    wq_p = pack_pairs(Wq)
    wk_p = pack_pairs(Wk)
    wqk_host = np.stack([wq_p, wk_p], axis=0).astype(bf)  # [2, pair, p, c, f]
    # wv: [p, c, pair, f]
    wv_host = np.ascontiguousarray(pack_pairs(Wv).transpose(1, 2, 0, 3)).astype(bf)

    return [
        {"xt": np.ascontiguousarray(xts[b]), "wqk": wqk_host, "wv": wv_host}
        for b in range(B)
    ]


def run_on_device(in_maps, **kwargs):
    nc = get_nc()
    return run_bass_kernel_spmd(nc, in_maps, list(range(B)), **kwargs)


def assemble(core_out):
    """[H, HS, T] out^T -> [T, H*HS]: pure layout transpose."""
    return np.ascontiguousarray(core_out.transpose(2, 0, 1).reshape(T, H * HS))


def kernel(x, Wq, Wk, Wv):
    in_maps = prep_inputs(x, Wq, Wk, Wv)
    res = run_on_device(in_maps)
    return np.stack([assemble(res.results[b]["out"]) for b in range(B)], axis=0)


# revision 18
# speedup vs baseline: 1.8756x; 1.8756x over previous
"""Multi-head causal attention (B=8, T=1024, C=1024, H=16, hs=64) on 8 trn2 cores.

Data-parallel over batch: core b computes full attention for x[b].

Device algorithm (per core), all matmuls bf16 inputs / fp32 PSUM accum:
  - xT [C, T] resident in SBUF (host pre-transposed, bf16), DMA'd per chunk
    interleaved with pair-0 Q/K weights so the PE starts ~1us in.
  - prologue: Q/K projection for pair 0 paced by the chunk DMAs, then the
    V projection for all heads (with a ones column at index 64 so the AV
    matmul also produces softmax denominators), interleaved with pair-0
    score blocks so ACT gets an early start.
  - scores transposed per (s_tile, head): scT[s_tile, t] = kT_chunk^T @ qT
    into a private [128,1024] PSUM tile, exp fused on ScalarE as ONE
    activation over the causal span (scale=1/8), diagonal 128x128 block
    masked by a tril multiply on GpSimd (keeps DVE/ACT free).
  - software-pipelined pair loop: iter p runs QK proj of pair p+2
    interleaved with score blocks of pair p+1 (covers PSUM-slot waits on
    the exp drain), then AV of pair p.
  - out^T[65, t] accumulated over s chunks: lhsT = [v | 1], rhs = expT.
    Row 64 = sum(exp) = softmax denominator. Normalize: copy PSUM->SBUF,
    DVE reciprocal on the [1,512] denominator row, GpSimd broadcast +
    multiply, DMA out.
"""

import numpy as np
import ml_dtypes

import concourse.bass as bass
import concourse.mybir as mybir
from concourse import bacc
from concourse.tile import TileContext
from concourse.bass import ds, ts
from concourse.bass_utils import run_bass_kernel_spmd
from concourse.masks import make_upper_triangular

BF16 = mybir.dt.bfloat16
F32 = mybir.dt.float32

B, T, C, H, HS = 8, 1024, 1024, 16, 64
P = 128
CK = C // P       # 8 contraction chunks
TT = T // P       # 8 t tiles
PAIRS = H // 2    # 8 head pairs
HALF = 512

_BUILT = None


def build_nc():
    nc = bacc.Bacc("TRN2", target_bir_lowering=False, debug=False)
    # [p, c, t] : xT[C, T] chunked; partition p, chunk c -> row 128c+p of xT
    xt = nc.dram_tensor("xt", [P, CK, T], BF16, kind="ExternalInput")
    # [proj(q,k), pair, p, c, f] : lhsT chunks, f = 2 heads x 64 stacked
    wqk = nc.dram_tensor("wqk", [2, PAIRS, P, CK, P], BF16, kind="ExternalInput")
    # [p, c, pair, f]
    wv = nc.dram_tensor("wv", [P, CK, PAIRS, P], BF16, kind="ExternalInput")
    # out^T per head: [head, d, t]; host transposes to [T, H*HS]
    out = nc.dram_tensor("out", [H, HS, T], F32, kind="ExternalOutput")

    with TileContext(nc) as tc:
        with (
            tc.tile_pool(name="const", bufs=1) as constp,
            tc.tile_pool(name="wpool", bufs=6) as wpool,
            tc.tile_pool(name="qkpool", bufs=6) as qkp,
            tc.tile_pool(name="exppool", bufs=16) as expp,
            tc.tile_pool(name="smallpool", bufs=4) as smallp,
            tc.tile_pool(name="ps1", bufs=2, space="PSUM") as ps1,
            tc.tile_pool(name="psSc", bufs=3, space="PSUM") as psSc,
        ):
            xt_sb = constp.tile([P, CK, T], BF16)
            # pair-group-major: [p, c, pg, 4*128] so the rhs slice is 2D
            wv_sb = constp.tile([P, CK, 2, 4 * P], BF16)

            # pair-0 weights first (2 whole DMAs), then xt per chunk on the
            # Sync queue; wv streams concurrently on the ACT queue so the V
            # phase isn't serialized behind xt.
            w_sb = {}
            w_sb[0] = (wpool.tile([P, CK, P], BF16, tag="w", name="wq0"),
                       wpool.tile([P, CK, P], BF16, tag="w", name="wk0"))
            nc.sync.dma_start(w_sb[0][0][:, :, :], wqk[0, 0, :, :, :])
            nc.sync.dma_start(w_sb[0][1][:, :, :], wqk[1, 0, :, :, :])
            for c in range(CK):
                nc.sync.dma_start(xt_sb[:, c, :], xt[:, c, :])
            for c in range(CK):
                nc.scalar.dma_start(
                    wv_sb[:, c, :, :],
                    wv[:, c, :, :].rearrange("p (g r) f -> p g (r f)", g=2),
                )

            def dma_w(pair):
                wq = wpool.tile([P, CK, P], BF16, tag="w", name=f"wq{pair}")
                wk = wpool.tile([P, CK, P], BF16, tag="w", name=f"wk{pair}")
                nc.sync.dma_start(wq[:, :, :], wqk[0, pair, :, :, :])
                nc.sync.dma_start(wk[:, :, :], wqk[1, pair, :, :, :])
                w_sb[pair] = (wq, wk)

            dma_w(1)
            dma_w(2)

            mask = constp.tile([P, P], BF16)
            make_upper_triangular(nc, mask, val=1.0, diag=True)
            # duplicated tril for masking both heads' diag blocks in one op
            mask2 = constp.tile([P, 2, P], BF16)
            for _w in range(2):
                nc.vector.tensor_copy(mask2[:, _w, :], mask[:, :])
            # [s_p, head, s_tile, 64 v cols + 1 ones col]
            v_all = constp.tile([P, H, TT, HS + 1], BF16)
            nc.gpsimd.memset(v_all[:, :, :, HS:HS + 1], 1.0)

            qk = {}

            def qk_half(pair, half_idx):
                """One of 4 projection halves: 8 chunk matmuls + PSUM->SBUF cast.
                half_idx: 0=q g0, 1=q g1, 2=k g0, 3=k g1."""
                if pair not in qk:
                    qk[pair] = (qkp.tile([P, T], BF16, tag="qk", name=f"q{pair}"),
                                qkp.tile([P, T], BF16, tag="qk", name=f"k{pair}"))
                proj, g = divmod(half_idx, 2)
                wsb = w_sb[pair][proj]
                dst = qk[pair][proj]
                pp = ps1.tile([P, HALF], F32, tag="ps", name=f"pp{pair}_{half_idx}")
                for c in range(CK):
                    nc.tensor.matmul(
                        pp[:, :],
                        wsb[:, c, :],
                        xt_sb[:, c, ds(HALF * g, HALF)],
                        start=(c == 0),
                        stop=(c == CK - 1),
                    )
                nc.vector.tensor_copy(dst[:, ds(HALF * g, HALF)], pp[:, :])

            es = {}

            def sc_block(pair, i):
                """Scores + exp + causal mask for s-tile i, both heads.

                Per head: private [128, T] PSUM tile, matmuls for the causal
                span [(t0,512),(512,1024)] (or single span when t0>=512),
                then ONE exp over [t0, 1024) and a tril mask-mult on GpSimd
                for the diagonal block. The two heads' matmuls alternate so
                they pair up in PE row groups (0,0)/(64,0)."""
                qT, kT = qk[pair]
                t0 = P * i
                e2 = expp.tile([P, 2, T], BF16, tag="exp", name=f"e{pair}_{i}")
                es[(pair, i)] = e2
                scs = []
                for w in range(2):
                    scs.append(psSc.tile([P, T], F32, tag="sc",
                                         name=f"sc{pair}_{i}_{w}"))
                spans = [(t0, HALF), (HALF, T)] if t0 < HALF else [(t0, T)]
                for a, b in spans:
                    for w in range(2):
                        po = HS * w
                        nc.tensor.matmul(
                            scs[w][:, ds(a, b - a)],
                            kT[ds(po, HS), ds(t0, P)],
                            qT[ds(po, HS), ds(a, b - a)],
                        )
                for w in range(2):
                    nc.scalar.activation(
                        e2[:, w, ds(t0, T - t0)],
                        scs[w][:, ds(t0, T - t0)],
                        mybir.ActivationFunctionType.Exp,
                        scale=HS ** -0.5,
                    )
                # single tril mask-mult covering both heads' diagonal
                # blocks. On DVE: GpSimd must stay single-op-type
                # (partition_broadcast) -- switching Q7 programs costs ~7us.
                nc.vector.tensor_tensor(
                    e2[:, :, ds(t0, P)], e2[:, :, ds(t0, P)],
                    mask2[:, :, :], mybir.AluOpType.mult,
                )

            def av_mm(pair, hh, w):
                """AV accumulation for head 2*pair+w, cols [512*hh, +512)."""
                h = 2 * pair + w
                av = ps1.tile([HS + 1, HALF], F32, tag="ps", name=f"av{h}_{hh}")
                contrib = [i for i in range(TT) if P * i < HALF * (hh + 1)]
                for idx, i in enumerate(contrib):
                    g0 = max(HALF * hh, P * i)
                    g1 = HALF * (hh + 1)
                    nc.tensor.matmul(
                        av[:, ds(g0 - HALF * hh, g1 - g0)],
                        v_all[:, h, i, :],
                        es[(pair, i)][:, w, ds(g0, g1 - g0)],
                        start=(idx == 0),
                        stop=(idx == len(contrib) - 1),
                    )
                return av

            def av_normalize(pair, avls):
                """Stage-batched normalization of the pair's 4 AV halves.

                Per half: copy PSUM->SBUF, DMA-repartition the [1,512]
                denominator row to [128,4] (DVE reciprocal cost scales with
                the FREE size, so spread values across lanes), reciprocal,
                DMA back, GpSimd broadcast across partitions, DVE multiply,
                DMA out. All DMAs issue from the DVE queue right after their
                producers (no semaphore stalls on the Sync queue); stages
                are batched across the 4 halves so DMA latency hides behind
                the other halves' DVE work."""
                chains = []
                for (hh, w), av in avls:
                    h = 2 * pair + w
                    nm = f"{h}_{hh}"
                    avs = smallp.tile([HS + 1, HALF], F32, tag="avs",
                                      name=f"avs{nm}")
                    nc.vector.tensor_copy(avs[:, :], av[:, :])
                    den_t = smallp.tile([P, 4], F32, tag="dent",
                                        name=f"den{nm}")
                    nc.sync.dma_start(den_t[:, :], avs[HS:HS + 1, :])
                    chains.append((hh, w, h, nm, avs, den_t))
                rbs = []
                for hh, w, h, nm, avs, den_t in chains:
                    rec_t = smallp.tile([P, 4], F32, tag="rect",
                                        name=f"rec{nm}")
                    nc.vector.reciprocal(rec_t[:, :], den_t[:, :])
                    recip_row = smallp.tile([1, HALF], F32, tag="recip",
                                            name=f"recip{nm}")
                    nc.sync.dma_start(recip_row[:, :], rec_t[:, :])
                    rb = smallp.tile([HS, HALF], F32, tag="rb", name=f"rb{nm}")
                    nc.gpsimd.partition_broadcast(rb[:, :], recip_row[0:1, :])
                    rbs.append(rb)
                for (hh, w, h, nm, avs, den_t), rb in zip(chains, rbs):
                    osb = smallp.tile([HS, HALF], F32, tag="osb",
                                      name=f"osb{nm}")
                    nc.vector.tensor_tensor(
                        osb[:, :], avs[0:HS, :], rb[:, :], mybir.AluOpType.mult,
                    )
                    nc.sync.dma_start(out[h, :, ds(HALF * hh, HALF)],
                                        osb[:, :])

            def av_half(pair, hh, w, defer=None):
                av = av_mm(pair, hh, w)
                defer.append(((hh, w), av))

            # ---- prologue: QK proj pair 0 (paced by the xt chunk DMAs) ----
            for k in range(4):
                qk_half(0, k)

            # ---- V for all heads, pair-0 score blocks interleaved ----
            for j in range(TT):
                for pg in range(2):
                    pv = ps1.tile([P, HALF], F32, tag="ps", name=f"pv{j}_{pg}")
                    for c in range(CK):
                        nc.tensor.matmul(
                            pv[:, :],
                            xt_sb[:, c, ts(j, P)],
                            wv_sb[:, c, pg, :],
                            start=(c == 0),
                            stop=(c == CK - 1),
                        )
                    # pv cols are (head0..head7 of group) x 64 in order
                    nc.vector.tensor_copy(
                        v_all[:, ds(8 * pg, 8), j, 0:HS],
                        pv.rearrange("p (g d) -> p g d", d=HS),
                    )
                sc_block(0, j)

            # ---- QK proj pair 1 ----
            for k in range(4):
                qk_half(1, k)

            # ---- software-pipelined pair loop ----
            for p in range(PAIRS):
                if p + 3 < PAIRS:
                    dma_w(p + 3)
                avls = []
                if p + 2 < PAIRS:
                    # QK proj of pair p+2 interleaved with scores of pair p+1
                    for k in range(4):
                        qk_half(p + 2, k)
                        sc_block(p + 1, 2 * k)
                        sc_block(p + 1, 2 * k + 1)
                    for hh in range(2):
                        for w in range(2):
                            av_half(p, hh, w, defer=avls)
                elif p + 1 < PAIRS:
                    # p == 6: no QK8; interleave SC7 with AV6 instead
                    order = [("av", 0, 0), ("sc", 0, 1), ("av", 0, 1),
                             ("sc", 2, 3), ("av", 1, 0), ("sc", 4, 5),
                             ("av", 1, 1), ("sc", 6, 7)]
                    for kind, a0, a1 in order:
                        if kind == "av":
                            av_half(p, a0, a1, defer=avls)
                        else:
                            sc_block(p + 1, a0)
                            sc_block(p + 1, a1)
                else:
                    for hh in range(2):
                        for w in range(2):
                            av_half(p, hh, w, defer=avls)
                av_normalize(p, avls)
    nc.compile()
    return nc


def get_nc():
    global _BUILT
    if _BUILT is None:
        _BUILT = build_nc()
    return _BUILT


def prep_inputs(x, Wq, Wk, Wv):
    """Host-side shard + layout prep. Returns in_maps (one dict per core)."""
    x = np.asarray(x, dtype=np.float32)
    Wq = np.asarray(Wq, dtype=np.float32)
    Wk = np.asarray(Wk, dtype=np.float32)
    Wv = np.asarray(Wv, dtype=np.float32)
    bf = ml_dtypes.bfloat16

    # xT[b]: [C, T] -> [p, c, t] with row 128c+p
    xts = []
    for b in range(B):
        xT = np.ascontiguousarray(x[b].T)          # [C, T]
        xts.append(xT.reshape(CK, P, T).transpose(1, 0, 2).astype(bf))

    def pack_pairs(W):
        # [H, C, hs] -> [pair, C, 128] -> [pair, p, c, f]
        Wp = W.reshape(PAIRS, 2, C, HS).transpose(0, 2, 1, 3).reshape(PAIRS, C, P)
        return Wp.reshape(PAIRS, CK, P, P).transpose(0, 2, 1, 3)  # [pair, p, c, f]

    wq_p = pack_pairs(Wq)
    wk_p = pack_pairs(Wk)
    wqk_host = np.stack([wq_p, wk_p], axis=0).astype(bf)  # [2, pair, p, c, f]
    # wv: [p, c, pair, f]
    wv_host = np.ascontiguousarray(pack_pairs(Wv).transpose(1, 2, 0, 3)).astype(bf)

    return [
        {"xt": np.ascontiguousarray(xts[b]), "wqk": wqk_host, "wv": wv_host}
        for b in range(B)
    ]


def run_on_device(in_maps, **kwargs):
    nc = get_nc()
    return run_bass_kernel_spmd(nc, in_maps, list(range(B)), **kwargs)


def assemble(core_out):
    """[H, HS, T] out^T -> [T, H*HS]: pure layout transpose."""
    return np.ascontiguousarray(core_out.transpose(2, 0, 1).reshape(T, H * HS))


def kernel(x, Wq, Wk, Wv):
    in_maps = prep_inputs(x, Wq, Wk, Wv)
    res = run_on_device(in_maps)
    return np.stack([assemble(res.results[b]["out"]) for b in range(B)], axis=0)


# revision 19
# speedup vs baseline: 2.2564x; 1.2031x over previous
"""Multi-head causal attention (B=8, T=1024, C=1024, H=16, hs=64) on 8 trn2 cores.

Data-parallel over batch: core b computes full attention for x[b].

Device algorithm (per core), all matmuls bf16 inputs / fp32 PSUM accum:
  - xT [C, T] resident in SBUF (host pre-transposed, bf16), DMA'd per chunk
    interleaved with pair-0 Q/K weights so the PE starts ~1us in.
  - prologue: Q/K projection for pair 0 paced by the chunk DMAs, then the
    V projection for all heads (with a ones column at index 64 so the AV
    matmul also produces softmax denominators), interleaved with pair-0
    score blocks so ACT gets an early start.
  - scores transposed per (s_tile, head): scT[s_tile, t] = kT_chunk^T @ qT
    into a private [128,1024] PSUM tile, exp fused on ScalarE as ONE
    activation over the causal span (scale=1/8), diagonal 128x128 block
    masked by a tril multiply on GpSimd (keeps DVE/ACT free).
  - software-pipelined pair loop: iter p runs QK proj of pair p+2
    interleaved with score blocks of pair p+1 (covers PSUM-slot waits on
    the exp drain), then AV of pair p.
  - out^T[65, t] accumulated over s chunks: lhsT = [v | 1], rhs = expT.
    Row 64 = sum(exp) = softmax denominator. Normalize: copy PSUM->SBUF,
    DVE reciprocal on the [1,512] denominator row, GpSimd broadcast +
    multiply, DMA out.
"""

import numpy as np
import ml_dtypes

import concourse.bass as bass
import concourse.mybir as mybir
from concourse import bacc
from concourse.tile import TileContext
from concourse.bass import ds, ts
from concourse.bass_utils import run_bass_kernel_spmd
from concourse.masks import make_upper_triangular

BF16 = mybir.dt.bfloat16
F32 = mybir.dt.float32

B, T, C, H, HS = 8, 1024, 1024, 16, 64
P = 128
CK = C // P       # 8 contraction chunks
TT = T // P       # 8 t tiles
PAIRS = H // 2    # 8 head pairs
HALF = 512

_BUILT = None


def build_nc():
    nc = bacc.Bacc("TRN2", target_bir_lowering=False, debug=False)
    # [p, c, t] : xT[C, T] chunked; partition p, chunk c -> row 128c+p of xT
    xt = nc.dram_tensor("xt", [P, CK, T], BF16, kind="ExternalInput")
    # [proj(q,k), pair, p, c, f] : lhsT chunks, f = 2 heads x 64 stacked
    wqk = nc.dram_tensor("wqk", [2, PAIRS, P, CK, P], BF16, kind="ExternalInput")
    # [p, c, pair, f]
    wv = nc.dram_tensor("wv", [P, CK, PAIRS, P], BF16, kind="ExternalInput")
    # out^T per head: [head, d, t]; host transposes to [T, H*HS]
    out = nc.dram_tensor("out", [H, HS, T], F32, kind="ExternalOutput")

    with TileContext(nc) as tc:
        with (
            tc.tile_pool(name="const", bufs=1) as constp,
            tc.tile_pool(name="wpool", bufs=6) as wpool,
            tc.tile_pool(name="qkpool", bufs=6) as qkp,
            tc.tile_pool(name="exppool", bufs=16) as expp,
            tc.tile_pool(name="smallpool", bufs=4) as smallp,
            tc.tile_pool(name="ps1", bufs=3, space="PSUM") as ps1,
            tc.tile_pool(name="psSc", bufs=2, space="PSUM") as psSc,
        ):
            xt_sb = constp.tile([P, CK, T], BF16)
            # pair-group-major: [p, c, pg, 4*128] so the rhs slice is 2D
            wv_sb = constp.tile([P, CK, 2, 4 * P], BF16)

            # pair-0 weights first (2 whole DMAs), then xt per chunk on the
            # Sync queue; wv streams concurrently on the ACT queue so the V
            # phase isn't serialized behind xt.
            w_sb = {}
            w_sb[0] = (wpool.tile([P, CK, P], BF16, tag="w", name="wq0"),
                       wpool.tile([P, CK, P], BF16, tag="w", name="wk0"))
            nc.sync.dma_start(w_sb[0][0][:, :, :], wqk[0, 0, :, :, :])
            nc.sync.dma_start(w_sb[0][1][:, :, :], wqk[1, 0, :, :, :])
            for c in range(CK):
                nc.sync.dma_start(xt_sb[:, c, :], xt[:, c, :])
            for c in range(CK):
                nc.scalar.dma_start(
                    wv_sb[:, c, :, :],
                    wv[:, c, :, :].rearrange("p (g r) f -> p g (r f)", g=2),
                )

            def dma_w(pair):
                wq = wpool.tile([P, CK, P], BF16, tag="w", name=f"wq{pair}")
                wk = wpool.tile([P, CK, P], BF16, tag="w", name=f"wk{pair}")
                nc.sync.dma_start(wq[:, :, :], wqk[0, pair, :, :, :])
                nc.sync.dma_start(wk[:, :, :], wqk[1, pair, :, :, :])
                w_sb[pair] = (wq, wk)

            dma_w(1)
            dma_w(2)

            mask = constp.tile([P, P], BF16)
            make_upper_triangular(nc, mask, val=1.0, diag=True)
            # duplicated tril for masking both heads' diag blocks in one op
            mask2 = constp.tile([P, 2, P], BF16)
            for _w in range(2):
                nc.vector.tensor_copy(mask2[:, _w, :], mask[:, :])
            # [s_p, head, s_tile, 64 v cols + 1 ones col]
            v_all = constp.tile([P, H, TT, HS + 1], BF16)
            nc.gpsimd.memset(v_all[:, :, :, HS:HS + 1], 1.0)

            qk = {}

            def qk_half(pair, half_idx):
                """One of 4 projection halves: 8 chunk matmuls + PSUM->SBUF cast.
                half_idx: 0=q g0, 1=q g1, 2=k g0, 3=k g1."""
                if pair not in qk:
                    qk[pair] = (qkp.tile([P, T], BF16, tag="qk", name=f"q{pair}"),
                                qkp.tile([P, T], BF16, tag="qk", name=f"k{pair}"))
                proj, g = divmod(half_idx, 2)
                wsb = w_sb[pair][proj]
                dst = qk[pair][proj]
                pp = ps1.tile([P, HALF], F32, tag="ps", name=f"pp{pair}_{half_idx}")
                for c in range(CK):
                    nc.tensor.matmul(
                        pp[:, :],
                        wsb[:, c, :],
                        xt_sb[:, c, ds(HALF * g, HALF)],
                        start=(c == 0),
                        stop=(c == CK - 1),
                    )
                nc.vector.tensor_copy(dst[:, ds(HALF * g, HALF)], pp[:, :])

            es = {}

            def sc_block(pair, i):
                """Scores + exp + causal mask for s-tile i, both heads.

                Per head: private [128, T] PSUM tile, matmuls for the causal
                span [(t0,512),(512,1024)] (or single span when t0>=512),
                then ONE exp over [t0, 1024) and a tril mask-mult on GpSimd
                for the diagonal block. The two heads' matmuls alternate so
                they pair up in PE row groups (0,0)/(64,0)."""
                qT, kT = qk[pair]
                t0 = P * i
                e2 = expp.tile([P, 2, T], BF16, tag="exp", name=f"e{pair}_{i}")
                es[(pair, i)] = e2
                scs = []
                for w in range(2):
                    scs.append(psSc.tile([P, T], F32, tag="sc",
                                         name=f"sc{pair}_{i}_{w}"))
                spans = [(t0, HALF), (HALF, T)] if t0 < HALF else [(t0, T)]
                for a, b in spans:
                    for w in range(2):
                        po = HS * w
                        nc.tensor.matmul(
                            scs[w][:, ds(a, b - a)],
                            kT[ds(po, HS), ds(t0, P)],
                            qT[ds(po, HS), ds(a, b - a)],
                        )
                for w in range(2):
                    nc.scalar.activation(
                        e2[:, w, ds(t0, T - t0)],
                        scs[w][:, ds(t0, T - t0)],
                        mybir.ActivationFunctionType.Exp,
                        scale=HS ** -0.5,
                    )
                # single tril mask-mult covering both heads' diagonal
                # blocks. On DVE: GpSimd must stay single-op-type
                # (partition_broadcast) -- switching Q7 programs costs ~7us.
                nc.vector.tensor_tensor(
                    e2[:, :, ds(t0, P)], e2[:, :, ds(t0, P)],
                    mask2[:, :, :], mybir.AluOpType.mult,
                )

            def av_mm(pair, hh, w):
                """AV accumulation for head 2*pair+w, cols [512*hh, +512)."""
                h = 2 * pair + w
                av = ps1.tile([HS + 1, HALF], F32, tag="ps", name=f"av{h}_{hh}")
                contrib = [i for i in range(TT) if P * i < HALF * (hh + 1)]
                for idx, i in enumerate(contrib):
                    g0 = max(HALF * hh, P * i)
                    g1 = HALF * (hh + 1)
                    nc.tensor.matmul(
                        av[:, ds(g0 - HALF * hh, g1 - g0)],
                        v_all[:, h, i, :],
                        es[(pair, i)][:, w, ds(g0, g1 - g0)],
                        start=(idx == 0),
                        stop=(idx == len(contrib) - 1),
                    )
                return av

            def av_normalize(pair, avls):
                """Stage-batched normalization of the pair's 4 AV halves.

                Per half: copy PSUM->SBUF, DMA-repartition the [1,512]
                denominator row to [128,4] (DVE reciprocal cost scales with
                the FREE size, so spread values across lanes), reciprocal,
                DMA back, GpSimd broadcast across partitions, DVE multiply,
                DMA out. All DMAs issue from the DVE queue right after their
                producers (no semaphore stalls on the Sync queue); stages
                are batched across the 4 halves so DMA latency hides behind
                the other halves' DVE work."""
                chains = []
                for (hh, w), av in avls:
                    h = 2 * pair + w
                    nm = f"{h}_{hh}"
                    avs = smallp.tile([HS + 1, HALF], F32, tag="avs",
                                      name=f"avs{nm}")
                    nc.vector.tensor_copy(avs[:, :], av[:, :])
                    den_t = smallp.tile([P, 4], F32, tag="dent",
                                        name=f"den{nm}")
                    nc.sync.dma_start(den_t[:, :], avs[HS:HS + 1, :])
                    chains.append((hh, w, h, nm, avs, den_t))
                rbs = []
                for hh, w, h, nm, avs, den_t in chains:
                    rec_t = smallp.tile([P, 4], F32, tag="rect",
                                        name=f"rec{nm}")
                    nc.vector.reciprocal(rec_t[:, :], den_t[:, :])
                    recip_row = smallp.tile([1, HALF], F32, tag="recip",
                                            name=f"recip{nm}")
                    nc.sync.dma_start(recip_row[:, :], rec_t[:, :])
                    rb = smallp.tile([HS, HALF], F32, tag="rb", name=f"rb{nm}")
                    nc.gpsimd.partition_broadcast(rb[:, :], recip_row[0:1, :])
                    rbs.append(rb)
                for (hh, w, h, nm, avs, den_t), rb in zip(chains, rbs):
                    osb = smallp.tile([HS, HALF], F32, tag="osb",
                                      name=f"osb{nm}")
                    nc.vector.tensor_tensor(
                        osb[:, :], avs[0:HS, :], rb[:, :], mybir.AluOpType.mult,
                    )
                    nc.sync.dma_start(out[h, :, ds(HALF * hh, HALF)],
                                        osb[:, :])

            def av_half(pair, hh, w, defer=None):
                av = av_mm(pair, hh, w)
                defer.append(((hh, w), av))

            # ---- prologue: QK proj pair 0 (paced by the xt chunk DMAs) ----
            for k in range(4):
                qk_half(0, k)

            # ---- V for all heads, pair-0 score blocks interleaved ----
            for j in range(TT):
                for pg in range(2):
                    pv = ps1.tile([P, HALF], F32, tag="ps", name=f"pv{j}_{pg}")
                    for c in range(CK):
                        nc.tensor.matmul(
                            pv[:, :],
                            xt_sb[:, c, ts(j, P)],
                            wv_sb[:, c, pg, :],
                            start=(c == 0),
                            stop=(c == CK - 1),
                        )
                    # pv cols are (head0..head7 of group) x 64 in order
                    nc.vector.tensor_copy(
                        v_all[:, ds(8 * pg, 8), j, 0:HS],
                        pv.rearrange("p (g d) -> p g d", d=HS),
                    )
                sc_block(0, j)

            # ---- QK proj pair 1 ----
            for k in range(4):
                qk_half(1, k)

            # ---- software-pipelined pair loop ----
            for p in range(PAIRS):
                if p + 3 < PAIRS:
                    dma_w(p + 3)
                avls = []
                if p + 2 < PAIRS:
                    # QK proj of pair p+2 interleaved with scores of pair p+1
                    for k in range(4):
                        qk_half(p + 2, k)
                        sc_block(p + 1, 2 * k)
                        sc_block(p + 1, 2 * k + 1)
                    for hh in range(2):
                        for w in range(2):
                            av_half(p, hh, w, defer=avls)
                elif p + 1 < PAIRS:
                    # p == 6: no QK8; interleave SC7 with AV6 instead
                    order = [("av", 0, 0), ("sc", 0, 1), ("av", 0, 1),
                             ("sc", 2, 3), ("av", 1, 0), ("sc", 4, 5),
                             ("av", 1, 1), ("sc", 6, 7)]
                    for kind, a0, a1 in order:
                        if kind == "av":
                            av_half(p, a0, a1, defer=avls)
                        else:
                            sc_block(p + 1, a0)
                            sc_block(p + 1, a1)
                else:
                    for hh in range(2):
                        for w in range(2):
                            av_half(p, hh, w, defer=avls)
                av_normalize(p, avls)
    nc.compile()
    return nc


def get_nc():
    global _BUILT
    if _BUILT is None:
        _BUILT = build_nc()
    return _BUILT


def prep_inputs(x, Wq, Wk, Wv):
    """Host-side shard + layout prep. Returns in_maps (one dict per core)."""
    x = np.asarray(x, dtype=np.float32)
    Wq = np.asarray(Wq, dtype=np.float32)
    Wk = np.asarray(Wk, dtype=np.float32)
    Wv = np.asarray(Wv, dtype=np.float32)
    bf = ml_dtypes.bfloat16

    # xT[b]: [C, T] -> [p, c, t] with row 128c+p
    xts = []
    for b in range(B):
        xT = np.ascontiguousarray(x[b].T)          # [C, T]
        xts.append(xT.reshape(CK, P, T).transpose(1, 0, 2).astype(bf))

    def pack_pairs(W):
        # [H, C, hs] -> [pair, C, 128] -> [pair, p, c, f]
        Wp = W.reshape(PAIRS, 2, C, HS).transpose(0, 2, 1, 3).reshape(PAIRS, C, P)
        return Wp.reshape(PAIRS, CK, P, P).transpose(0, 2, 1, 3)  # [pair, p, c, f]

    wq_p = pack_pairs(Wq)
    wk_p = pack_pairs(Wk)
    wqk_host = np.stack([wq_p, wk_p], axis=0).astype(bf)  # [2, pair, p, c, f]
    # wv: [p, c, pair, f]
    wv_host = np.ascontiguousarray(pack_pairs(Wv).transpose(1, 2, 0, 3)).astype(bf)

    return [
        {"xt": np.ascontiguousarray(xts[b]), "wqk": wqk_host, "wv": wv_host}
        for b in range(B)
    ]


def run_on_device(in_maps, **kwargs):
    nc = get_nc()
    return run_bass_kernel_spmd(nc, in_maps, list(range(B)), **kwargs)


def assemble(core_out):
    """[H, HS, T] out^T -> [T, H*HS]: pure layout transpose."""
    return np.ascontiguousarray(core_out.transpose(2, 0, 1).reshape(T, H * HS))


def kernel(x, Wq, Wk, Wv):
    in_maps = prep_inputs(x, Wq, Wk, Wv)
    res = run_on_device(in_maps)
    return np.stack([assemble(res.results[b]["out"]) for b in range(B)], axis=0)
